# revision 1
# baseline (speedup 1.0000x reference)
"""DeepFM forward on Trainium2, 8 NeuronCores, data-parallel over batch.

Reference computes (B=512, n=512, K=4, H=128, n_pairs=130816):
    S  = fm_w @ fm_w.T
    fm = x[:, i1] * x[:, i2] * S[i1, i2]        # [B, n_pairs]
    h2 = relu(relu(x@w1+b1)@w2+b2)
    out = sigmoid(concat([fm, h2]) @ wo + bo)

The fm @ wo[:n_pairs] contraction is the bilinear form
    t1[b] = x[b]^T Wp' x[b]  with  Wp'[i,j] = S[i,j] * Wp[i,j]
where Wp is wo[:n_pairs] scattered into the strictly-upper triangle of a
[n, n] matrix (a pure re-layout of wo done on host; indices are static).
Since S = fm_w @ fm_w.T has rank 4, this factors as
    t1[b] = sum_t z_t[b]^T Wp z_t[b],  z_t = x * fm_w[:, t]
so the device never materializes S: Wp is used directly as the matmul
operand and the rank-4 scaling is cheap broadcast DVE work. Wp is
strictly upper triangular, so only the 10 upper-triangular 128x128
blocks are shipped and multiplied (the 6 lower blocks are zero).

All inputs are repacked on host into per-partition-contiguous [128, X]
SBUF images so each dma_start moves 128 fat contiguous runs (the SDMA
per-packet cost dominates latency otherwise). The critical small
tensors (x shard, fm_w, biases) ride one early DMA on the sync queue;
the f32 section lives in the bf16 image via bitcast.

Per-core program (batch shard = 64 columns, feature-on-partition layout,
bf16 operands / fp32 accumulation; t stacked along the free dim):
    Z_k[:, t, :]  = xT_k * fm_w[k-chunk, t]       (DVE broadcast mul)
    VT_j = sum_{k<=j} Wp[k128, j128]^T @ Z_k      (PE, j-major blocks)
    Q_j  = VT_j * Z_j                             (DVE, bf16 out)
    t    = sum_{j,t} Q_j[:,t,:]^T @ ones + h2^T @ wo_h  (PE psum accum) [64,1]
    h1   = max(w1^T @ xT + b1, 0)                 (PE+DVE)
    h2   = max(w2^T @ h1 + b2, 0)                 (PE+DVE)
    out  = sigmoid(t + bo)                        (ACT, table pre-warmed)

The PE is HAM-warmed with dummy matmuls on memset tiles during the DMA
wait so the back half of the kernel runs at the fast clock.
"""

import os
import sys

import numpy as np

for _p in ("/opt/trn_rl_repo", "/root/.axon_site/_ro/trn_rl_repo"):
    if os.path.isdir(_p) and _p not in sys.path:
        sys.path.insert(0, _p)

import ml_dtypes

import concourse.bass as bass
import concourse.tile as tile
from concourse import bacc, mybir
from concourse.bass import ts
from concourse.bass_utils import run_bass_kernel_spmd

F32 = mybir.dt.float32
BF16 = mybir.dt.bfloat16
AF = mybir.ActivationFunctionType
ALU = mybir.AluOpType

N = 512          # n_feat
KFM = 4          # fm embedding dim
H = 128          # mlp hidden
NP = N * (N - 1) // 2
B = 512
N_CORES = 8
BC = B // N_CORES  # 64 batch rows per core
NCH = N // 128     # 4 feature chunks
N_WARM = int(os.environ.get("DFM_N_WARM", "16"))  # PE warm-up dummy matmuls

# Upper-triangular 128x128 blocks of Wp in j-major order.
UBLOCKS = [(k, j) for j in range(NCH) for k in range(j + 1)]
UB_OFF = {kj: i * 128 for i, kj in enumerate(UBLOCKS)}  # column offset in image
WP_COLS = len(UBLOCKS) * 128  # 1280
WP_SPLIT = UB_OFF[(0, 2)]     # j0+j1 blocks first, then j2+j3's

# f32 pack layout (viewed at [128, 20] f32): [fmw (4*4) | b1 | b2 | woh | bo]
FM_OFF = 0
PK_OFF = FM_OFF + NCH * KFM
F32_COLS = PK_OFF + 4      # 20
# crit image (bf16): [xt (4*64) | f32 pack as raw bf16 pairs (40)]
XT_OFF = 0
FP_OFF = NCH * BC          # 256
CRIT_COLS = FP_OFF + F32_COLS * 2  # 296

_IU1, _IU2 = np.triu_indices(N, k=1)

_program_cache = None


def _chunk_pack(a, cols):
    """[512, cols] row-major -> [128, 4*cols] with chunk c at column block c."""
    return np.ascontiguousarray(
        a.reshape(NCH, 128, cols).transpose(1, 0, 2).reshape(128, NCH * cols)
    )


def _build_program():
    global _program_cache
    if _program_cache is not None:
        return _program_cache

    nc = bacc.Bacc(
        "TRN2", target_bir_lowering=False, debug=False, num_devices=N_CORES
    )
    crit_d = nc.declare_dram_parameter("crit", [128, CRIT_COLS], BF16, isOutput=False)
    wp_d = nc.declare_dram_parameter("wp", [128, WP_COLS], BF16, isOutput=False)
    w12_d = nc.declare_dram_parameter(
        "w12", [128, NCH * H + H], BF16, isOutput=False
    )
    out_d = nc.declare_dram_parameter("out", [1, BC], F32, isOutput=True)

    with tile.TileContext(nc) as tc:
        with (
            tc.tile_pool(name="const", bufs=1) as cpool,
            tc.tile_pool(name="work", bufs=1) as wpool,
            tc.tile_pool(name="ps_v", bufs=1, space=bass.MemorySpace.PSUM) as vpool,
            tc.tile_pool(name="ps_h", bufs=1, space=bass.MemorySpace.PSUM) as hpool,
            tc.tile_pool(name="ps_t", bufs=1, space=bass.MemorySpace.PSUM) as tpool,
        ):
            # ---- loads. sync queue: crit first, then the Wp halves ----
            crit_sb = cpool.tile([128, CRIT_COLS], BF16)
            nc.sync.dma_start(crit_sb[:], crit_d[:, :])
            wp_sb = cpool.tile([128, WP_COLS], BF16)
            s2, s3 = UB_OFF[(0, 2)], UB_OFF[(0, 3)]
            nc.sync.dma_start(wp_sb[:, :s2], wp_d[:, :s2])
            nc.sync.dma_start(wp_sb[:, s2:s3], wp_d[:, s2:s3])
            nc.sync.dma_start(wp_sb[:, s3:], wp_d[:, s3:])
            w12_sb = cpool.tile([128, NCH * H + H], BF16)
            nc.scalar.dma_start(w12_sb[:], w12_d[:, :])

            f32v = crit_sb[:, FP_OFF:].bitcast(F32)  # [128, 20] f32 view

            def xt(k):
                return crit_sb[:, XT_OFF + k * BC : XT_OFF + (k + 1) * BC]

            def w1c(k):
                return w12_sb[:, k * H : (k + 1) * H]

            w2_ap = w12_sb[:, NCH * H : NCH * H + H]
            b1_ap = f32v[:, PK_OFF : PK_OFF + 1]
            b2_ap = f32v[:, PK_OFF + 1 : PK_OFF + 2]
            woh_ap = f32v[:, PK_OFF + 2 : PK_OFF + 3]
            bo_ap = f32v[0:1, PK_OFF + 3 : PK_OFF + 4]

            # ---- constants (Vector memsets — fast, idle early) ----
            dum_lhs = cpool.tile([128, 128], BF16)
            nc.vector.memset(dum_lhs[:], 0.0)
            dum_rhs = cpool.tile([128, KFM * BC], BF16)
            nc.vector.memset(dum_rhs[:], 0.0)
            ones_sb = cpool.tile([128, 1], BF16)
            nc.vector.memset(ones_sb[:], 1.0)
            warm_in = cpool.tile([1, 1], F32)
            nc.vector.memset(warm_in[:], 0.0)
            warm_out = cpool.tile([1, 1], F32)
            nc.scalar.activation(warm_out[:], warm_in[:], AF.Sigmoid, bias=0.0)

            # ---- PE HAM warm-up into the (late-used) MLP/t psum banks ----
            dum_tags = ["h1_ps", "h2_ps", "t_ps"]
            for d in range(N_WARM):
                dum_ps = hpool.tile(
                    [128, KFM * BC], F32, name=f"dum{d}",
                    tag=dum_tags[d % 2],
                )
                nc.tensor.matmul(
                    dum_ps[:], dum_lhs[:], dum_rhs[:], start=True, stop=True
                )

            # ---- Z_k[:, t, :] = xT_k scaled by fm_w column t (rank-4) ----
            z_tiles = []
            for k in range(NCH):
                z_sb = wpool.tile([128, KFM, BC], BF16, name=f"z{k}", tag=f"z{k}")
                nc.vector.tensor_mul(
                    z_sb[:],
                    xt(k)[:, None, :].broadcast_to([128, KFM, BC]),
                    f32v[:, FM_OFF + k * KFM : FM_OFF + (k + 1) * KFM][
                        :, :, None
                    ].broadcast_to([128, KFM, BC]),
                )
                z_tiles.append(z_sb)

            # ---- VT_j = sum_{k<=j} Wp[k,j]^T @ Z_k (upper blocks only) ----
            vt_tiles = [
                vpool.tile([128, KFM, BC], F32, name=f"vt{j}", tag=f"v{j}")
                for j in range(NCH)
            ]
            for j in range(NCH):
                for k in range(j + 1):
                    off = UB_OFF[(k, j)]
                    nc.tensor.matmul(
                        vt_tiles[j][:], wp_sb[:, off : off + 128], z_tiles[k][:],
                        start=(k == 0), stop=(k == j),
                    )

            # ---- MLP ----
            h1_ps = hpool.tile([H, BC], F32)
            for k in range(NCH):
                nc.tensor.matmul(
                    h1_ps[:], w1c(k), xt(k),
                    start=(k == 0), stop=(k == NCH - 1),
                )
            h1_sb = wpool.tile([H, BC], BF16)
            nc.vector.tensor_scalar(
                h1_sb[:], h1_ps[:], b1_ap, 0.0, op0=ALU.add, op1=ALU.max
            )
            h2_ps = hpool.tile([H, BC], F32)
            nc.tensor.matmul(h2_ps[:], w2_ap, h1_sb[:], start=True, stop=True)
            h2_sb = wpool.tile([H, BC], F32)
            nc.vector.tensor_scalar(
                h2_sb[:], h2_ps[:], b2_ap, 0.0, op0=ALU.add, op1=ALU.max
            )

            # ---- Q_j = VT_j * Z_j; fold partitions and t into t_ps [1, 64] ----
            t_ps = tpool.tile([1, BC], F32, tag="t_ps")
            for j in range(NCH):
                q_sb = wpool.tile([128, KFM, BC], BF16, name=f"q{j}", tag=f"q{j}")
                nc.vector.tensor_mul(q_sb[:], vt_tiles[j][:], z_tiles[j][:])
                for t in range(KFM):
                    nc.tensor.matmul(
                        t_ps[:], ones_sb[:], q_sb[:, t, :],
                        start=(j == 0 and t == 0), stop=False,
                    )
            nc.tensor.matmul(t_ps[:], woh_ap, h2_sb[:], start=False, stop=True)

            out_sb = wpool.tile([1, BC], F32)
            nc.scalar.activation(out_sb[:], t_ps[:], AF.Sigmoid, bias=bo_ap)
            nc.scalar.dma_start(out_d[:, :], out_sb[:])

    nc.compile()
    _program_cache = nc
    return nc


def _prep_inputs(x, fm_w, w1, b1, w2, b2, wo, bo):
    x = np.asarray(x, dtype=np.float32)
    fm_w = np.asarray(fm_w, dtype=np.float32)
    w1 = np.asarray(w1, dtype=np.float32)
    w2 = np.asarray(w2, dtype=np.float32)
    wo = np.asarray(wo, dtype=np.float32).reshape(NP + H)
    b1 = np.asarray(b1, dtype=np.float32).reshape(H)
    b2 = np.asarray(b2, dtype=np.float32).reshape(H)
    bo = np.asarray(bo, dtype=np.float32).reshape(1)

    bf = ml_dtypes.bfloat16

    # Scatter pair weights into the strictly-upper triangle (static index
    # relayout, same (j1, j2>j1) row-major order as the reference), then
    # pack only the upper-triangular 128x128 blocks, j-major.
    wp = np.zeros((N, N), dtype=np.float32)
    wp[_IU1, _IU2] = wo[:NP]
    wp_bf = wp.astype(bf)
    wp_img = np.empty((128, WP_COLS), dtype=bf)
    for (k, j), off in UB_OFF.items():
        wp_img[:, off : off + 128] = wp_bf[
            128 * k : 128 * (k + 1), 128 * j : 128 * (j + 1)
        ]
    wp_img = np.ascontiguousarray(wp_img)

    w12_img = np.empty((128, NCH * H + H), dtype=bf)
    w12_img[:, : NCH * H] = _chunk_pack(w1.astype(bf), H)
    w12_img[:, NCH * H :] = w2.astype(bf)
    w12_img = np.ascontiguousarray(w12_img)

    f32_img = np.zeros((128, F32_COLS), dtype=np.float32)
    f32_img[:, FM_OFF : FM_OFF + NCH * KFM] = _chunk_pack(fm_w, KFM)
    f32_img[:, PK_OFF] = b1
    f32_img[:, PK_OFF + 1] = b2
    f32_img[:, PK_OFF + 2] = wo[NP:]
    f32_img[:, PK_OFF + 3] = bo[0]   # replicated: per-partition sigmoid bias

    xT = x.T.astype(bf)                                         # [512, 512]

    in_maps = []
    for c in range(N_CORES):
        crit = np.empty((128, CRIT_COLS), dtype=bf)
        crit[:, XT_OFF:FP_OFF] = _chunk_pack(
            np.ascontiguousarray(xT[:, c * BC : (c + 1) * BC]), BC
        )
        crit[:, FP_OFF:] = f32_img.view(bf)   # raw f32 bytes as bf16 pairs
        in_maps.append(
            {
                "crit": np.ascontiguousarray(crit),
                "wp": wp_img,
                "w12": w12_img,
            }
        )
    return in_maps


def run(inputs, **spmd_kwargs):
    """Build, run on 8 cores, return (output [512,1] f32, BassKernelResults)."""
    nc = _build_program()
    in_maps = _prep_inputs(**inputs)
    res = run_bass_kernel_spmd(nc, in_maps, list(range(N_CORES)), **spmd_kwargs)
    out = np.concatenate(
        [res.results[c]["out"].reshape(BC) for c in range(N_CORES)]
    ).reshape(B, 1).astype(np.float32)
    return out, res


def kernel(**inputs) -> np.ndarray:
    out, _ = run(inputs)
    return out



# revision 5
# speedup vs baseline: 1.0841x; 1.0841x over previous
"""DeepFM forward on Trainium2, 8 NeuronCores, data-parallel over batch.

Reference computes (B=512, n=512, K=4, H=128, n_pairs=130816):
    S  = fm_w @ fm_w.T
    fm = x[:, i1] * x[:, i2] * S[i1, i2]        # [B, n_pairs]
    h2 = relu(relu(x@w1+b1)@w2+b2)
    out = sigmoid(concat([fm, h2]) @ wo + bo)

The fm @ wo[:n_pairs] contraction is the bilinear form
    t1[b] = x[b]^T Wp' x[b]  with  Wp'[i,j] = S[i,j] * Wp[i,j]
where Wp is wo[:n_pairs] scattered into the strictly-upper triangle of a
[n, n] matrix (host-side static relayout). S = fm_w fm_w^T has rank 4, so
    t1[b] = sum_t z_t[b]^T Wp z_t[b],  z_t = x * fm_w[:, t]
and only the 10 upper-triangular 128x128 blocks of Wp are shipped.

All matmul operands are fp8e4 (TRN e4m3, +-240) with power-of-2 scales
chosen so every tensor sits mid-range; the final sigmoid applies the
inverse scale. This halves HBM traffic vs bf16 (the DMA is the dominant
cost) with ~7e-4 relative error (threshold 2e-2).

Everything lives in the [batch, t] free layout so the final fold over t
is one Vector tensor_reduce. Per-core program (BC=64 batch cols):
    z_k[:, b, t] = xT_k * fmw16[k][:, t]        (DVE/Pool, fp8 out)
    VT_j = sum_{k<=j} Wp[k,j]^T @ z_k           (PE, fp8, psum f32)
    Q_j  = VT_j * z_j                           (DVE, bf16)
    t_ps[1, b, t] += ones^T @ Q_j               (PE)
    h1 = relu8(w1^T xT + 16 b1); h2 = relu16(w2^T h1 + 32 b2)  (PE+ACT)
    deep_ps = (2048 woh)^T @ h2                 (PE)
    t = reduce_t(t_ps); out = sigmoid((t + deep)/65536 + bo)   (DVE+ACT)

DMA plan (fixed ~700ns per dma_start on the issuing queue; transfers
share the SDMA rings): sync carries critA (x, fm_w, biases, wp00) then
critB (wp j2/j3 blocks); scalar carries critC (w1, w2, wp j1) in
parallel. PE is HAM-warmed with dummy fp8 matmuls during the DMA wait,
and a few post-output dummy ops keep the clock up into the teardown.
"""

import os
import sys

import numpy as np

for _p in ("/opt/trn_rl_repo", "/root/.axon_site/_ro/trn_rl_repo"):
    if os.path.isdir(_p) and _p not in sys.path:
        sys.path.insert(0, _p)

import ml_dtypes

import concourse.bass as bass
import concourse.tile as tile
from concourse import bacc, mybir
from concourse.bass_utils import run_bass_kernel_spmd

F32 = mybir.dt.float32
BF16 = mybir.dt.bfloat16
FP8 = mybir.dt.float8e4
AF = mybir.ActivationFunctionType
ALU = mybir.AluOpType

N = 512          # n_feat
KFM = 4          # fm embedding dim
H = 128          # mlp hidden
NP = N * (N - 1) // 2
B = 512
N_CORES = 8
BC = B // N_CORES  # 64 batch rows per core
NCH = N // 128     # 4 feature chunks

# fp8 scales (powers of two)
S_FMW = 16.0
S_WP = 256.0
S_W1 = 16.0
S_W2 = 2.0
S_T = S_FMW * S_FMW * S_WP          # 65536 on both t contributions
S_WOH = S_T / (S_W1 * S_W2)         # 2048

N_WARM = int(os.environ.get("DFM_N_WARM", "6"))
RELU_ENG = os.environ.get("DFM_RELU_ENG", "scalar")
Z3_ENG = os.environ.get("DFM_Z3_ENG", "gpsimd")
HAM_TAIL = int(os.environ.get("DFM_HAM_TAIL", "2"))

# critA image (fp8 bytes): [ xT (4*64) | f32 pack (19*4) | woh bf16 | ones bf16 | wp(0,0) ]
XT_OFF = 0
F32_OFF = NCH * BC               # 256
F32_COLS = NCH * KFM + 3         # fmw16 | b1*16 | b2*32 | bo  -> 19
BF_OFF = F32_OFF + F32_COLS * 4  # 332
WPA_OFF = BF_OFF + 4             # 336
A_COLS = WPA_OFF + 128           # 464
PK_B1 = NCH * KFM                # 16
PK_B2 = PK_B1 + 1
PK_BO = PK_B2 + 1

# critC: [ w1*16 (4*128) | w2*2 (128) | wp(0,1) | wp(1,1) ]
C_W2 = NCH * H                   # 512
C_WP0 = C_W2 + H                 # 640
C_COLS = C_WP0 + 2 * 128         # 896

# critB: wp blocks (0,2),(1,2),(2,2),(0,3),(1,3),(2,3),(3,3)
B_BLOCKS = [(0, 2), (1, 2), (2, 2), (0, 3), (1, 3), (2, 3), (3, 3)]
B_COLS = len(B_BLOCKS) * 128     # 896

_IU1, _IU2 = np.triu_indices(N, k=1)

_program_cache = None


def _build_program():
    global _program_cache
    if _program_cache is not None:
        return _program_cache

    nc = bacc.Bacc(
        "TRN2", target_bir_lowering=False, debug=False, num_devices=N_CORES
    )
    critA_d = nc.declare_dram_parameter("critA", [128, A_COLS], FP8, isOutput=False)
    critB_d = nc.declare_dram_parameter("critB", [128, B_COLS], FP8, isOutput=False)
    critC_d = nc.declare_dram_parameter("critC", [128, C_COLS], FP8, isOutput=False)
    out_d = nc.declare_dram_parameter("out", [1, BC], F32, isOutput=True)

    relu_eng_is_scalar = RELU_ENG == "scalar"

    with tile.TileContext(nc) as tc:
        with (
            tc.tile_pool(name="const", bufs=1) as cpool,
            tc.tile_pool(name="work", bufs=1) as wpool,
            tc.tile_pool(name="ps_v", bufs=1, space=bass.MemorySpace.PSUM) as vpool,
            tc.tile_pool(name="ps_h", bufs=1, space=bass.MemorySpace.PSUM) as hpool,
            tc.tile_pool(name="ps_t", bufs=1, space=bass.MemorySpace.PSUM) as tpool,
        ):
            # ---- input DMAs: sync gets A then B, scalar gets C ----
            critA_sb = cpool.tile([128, A_COLS], FP8)
            nc.sync.dma_start(critA_sb[:], critA_d[:, :])
            critC_sb = cpool.tile([128, C_COLS], FP8)
            nc.scalar.dma_start(critC_sb[:], critC_d[:, :])
            critB_sb = cpool.tile([128, B_COLS], FP8)
            nc.sync.dma_start(critB_sb[:], critB_d[:, :])

            # ---- views into critA ----
            f32v = critA_sb[:, F32_OFF:BF_OFF].bitcast(F32)       # [128, 19]
            bf16v = critA_sb[:, BF_OFF:WPA_OFF].bitcast(BF16)     # [128, 2]
            woh_ap = bf16v[:, 0:1]
            ones_ap = bf16v[:, 1:2]
            b1_ap = f32v[:, PK_B1 : PK_B1 + 1]
            b2_ap = f32v[:, PK_B2 : PK_B2 + 1]
            bo_ap = f32v[0:1, PK_BO : PK_BO + 1]

            def xt(k):
                return critA_sb[:, XT_OFF + k * BC : XT_OFF + (k + 1) * BC]

            def fmw(k):
                return f32v[:, k * KFM : (k + 1) * KFM]

            wpA = critA_sb[:, WPA_OFF : WPA_OFF + 128]

            def w1c(k):
                return critC_sb[:, k * H : (k + 1) * H]

            w2_ap = critC_sb[:, C_W2 : C_W2 + H]
            wp_blk = {(0, 0): wpA}
            wp_blk[(0, 1)] = critC_sb[:, C_WP0 : C_WP0 + 128]
            wp_blk[(1, 1)] = critC_sb[:, C_WP0 + 128 : C_WP0 + 256]
            for i, kj in enumerate(B_BLOCKS):
                wp_blk[kj] = critB_sb[:, i * 128 : (i + 1) * 128]

            # ---- warm tiles (GpSimd memsets; Vector stays free) ----
            dum_lhs = cpool.tile([128, 128], FP8)
            nc.gpsimd.memset(dum_lhs[:], 0.0)
            dum_rhs = cpool.tile([128, BC], FP8)
            nc.gpsimd.memset(dum_rhs[:], 0.0)
            warm_in = cpool.tile([1, 1], F32)
            nc.gpsimd.memset(warm_in[:], 0.0)

            h1_ps = hpool.tile([H, BC], F32, tag="h1")
            for d in range(N_WARM):
                nc.tensor.matmul(
                    h1_ps[:], dum_lhs[:], dum_rhs[:], start=True, stop=True
                )
            warm_out = cpool.tile([1, 1], F32)
            nc.scalar.activation(warm_out[:], warm_in[:], AF.Sigmoid, bias=0.0)

            # ---- z_k [128, BC, KFM] fp8: x column-scaled by fm_w (rank-4) ----
            z_all = wpool.tile([128, NCH, BC, KFM], FP8, name="z_all", tag="z")

            def z(k):
                return z_all[:, k]

            for k in range(NCH):
                eng = nc.vector if (k < 3 or Z3_ENG == "vector") else nc.gpsimd
                eng.tensor_mul(
                    z(k),
                    xt(k)[:, :, None].broadcast_to([128, BC, KFM]),
                    fmw(k)[:, None, :].broadcast_to([128, BC, KFM]),
                )

            # ---- psum tiles ----
            vt = [
                vpool.tile([128, BC, KFM], F32, name=f"vt{j}", tag=f"v{j}")
                for j in range(NCH)
            ]
            h2_ps = hpool.tile([H, BC], F32, tag="h2")
            t_ps = tpool.tile([1, BC, KFM], F32, tag="t")
            deep_ps = tpool.tile([1, BC], F32, tag="deep")

            q_all = wpool.tile([128, NCH, BC, KFM], BF16, name="q_all", tag="q")

            def q(j):
                return q_all[:, j]

            h1_sb = wpool.tile([H, BC], FP8, name="h1_sb")
            h2_sb = wpool.tile([H, BC], BF16, name="h2_sb")

            def relu(dst, src, bias_ap):
                if relu_eng_is_scalar:
                    nc.scalar.activation(dst, src, AF.Relu, bias=bias_ap)
                else:
                    nc.vector.tensor_scalar(
                        dst, src, bias_ap, 0.0, op0=ALU.add, op1=ALU.max
                    )

            # ---- PE stream interleaved with DVE/ACT consumers ----
            # VT j0 (needs critA only)
            nc.tensor.matmul(vt[0][:], wpA, z(0), start=True, stop=True)
            # h1 (needs critC)
            for k in range(NCH):
                nc.tensor.matmul(
                    h1_ps[:], w1c(k), xt(k), start=(k == 0), stop=(k == NCH - 1)
                )
            relu(h1_sb[:], h1_ps[:], b1_ap)
            # VT j1
            nc.tensor.matmul(vt[1][:], wp_blk[(0, 1)], z(0), start=True, stop=False)
            nc.tensor.matmul(vt[1][:], wp_blk[(1, 1)], z(1), start=False, stop=True)
            # Q0
            nc.vector.tensor_mul(q(0), vt[0][:], z(0))
            # h2
            nc.tensor.matmul(h2_ps[:], w2_ap, h1_sb[:], start=True, stop=True)
            relu(h2_sb[:], h2_ps[:], b2_ap)
            # VT j2
            for i, k in enumerate(range(3)):
                nc.tensor.matmul(
                    vt[2][:], wp_blk[(k, 2)], z(k),
                    start=(i == 0), stop=(i == 2), skip_group_check=True,
                )
            # Q1
            nc.vector.tensor_mul(q(1), vt[1][:], z(1))
            # t accumulation group start + deep
            nc.tensor.matmul(
                t_ps[:], ones_ap, q(0), start=True, stop=False,
                skip_group_check=True,
            )
            nc.tensor.matmul(
                deep_ps[:], woh_ap, h2_sb[:], start=True, stop=True,
                skip_group_check=True,
            )
            # VT j3
            for i, k in enumerate(range(4)):
                nc.tensor.matmul(
                    vt[3][:], wp_blk[(k, 3)], z(k),
                    start=(i == 0), stop=(i == 3), skip_group_check=True,
                )
            # Q2, t1, Q3, t2, t3
            nc.vector.tensor_mul(q(2), vt[2][:], z(2))
            nc.tensor.matmul(
                t_ps[:], ones_ap, q(1), start=False, stop=False,
                skip_group_check=True,
            )
            nc.vector.tensor_mul(q(3), vt[3][:], z(3))
            nc.tensor.matmul(
                t_ps[:], ones_ap, q(2), start=False, stop=False,
                skip_group_check=True,
            )
            nc.tensor.matmul(
                t_ps[:], ones_ap, q(3), start=False, stop=True,
                skip_group_check=True,
            )

            # ---- fold over t, add deep, sigmoid, store ----
            tsum_sb = wpool.tile([1, BC], F32, name="tsum")
            nc.vector.tensor_reduce(
                tsum_sb[:], t_ps[:], axis=mybir.AxisListType.X, op=ALU.add
            )
            tlog_sb = wpool.tile([1, BC], F32, name="tlog")
            nc.vector.tensor_add(tlog_sb[:], tsum_sb[:], deep_ps[:])
            out_sb = wpool.tile([1, BC], F32, name="out_sb")
            nc.scalar.activation(
                out_sb[:], tlog_sb[:], AF.Sigmoid, bias=bo_ap, scale=1.0 / S_T
            )
            nc.sync.dma_start(out_d[:, :], out_sb[:])

            # ---- keep HAM clock high into the teardown sweep ----
            for _ in range(HAM_TAIL):
                nc.tensor.matmul(
                    h1_ps[:], dum_lhs[:], dum_rhs[:], start=True, stop=True
                )

    nc.compile()
    _program_cache = nc
    return nc


def _q8(a, scale):
    return np.clip(
        np.asarray(a, np.float32) * scale, -240.0, 240.0
    ).astype(ml_dtypes.float8_e4m3fn)


def _chunk_pack(a, cols):
    """[512, cols] row-major -> [128, 4, cols] -> [128, 4*cols]."""
    return np.ascontiguousarray(
        a.reshape(NCH, 128, cols).transpose(1, 0, 2).reshape(128, NCH * cols)
    )


def _prep_inputs(x, fm_w, w1, b1, w2, b2, wo, bo):
    x = np.asarray(x, dtype=np.float32)
    fm_w = np.asarray(fm_w, dtype=np.float32)
    w1 = np.asarray(w1, dtype=np.float32)
    w2 = np.asarray(w2, dtype=np.float32)
    wo = np.asarray(wo, dtype=np.float32).reshape(NP + H)
    b1 = np.asarray(b1, dtype=np.float32).reshape(H)
    b2 = np.asarray(b2, dtype=np.float32).reshape(H)
    bo = np.asarray(bo, dtype=np.float32).reshape(1)

    fp8 = ml_dtypes.float8_e4m3fn
    bf = ml_dtypes.bfloat16

    # Pair weights scattered into the strictly-upper triangle, fp8-scaled.
    wp = np.zeros((N, N), dtype=np.float32)
    wp[_IU1, _IU2] = wo[:NP]
    wp_q = _q8(wp, S_WP)

    def blk(k, j):
        return wp_q[128 * k : 128 * (k + 1), 128 * j : 128 * (j + 1)]

    critB = np.empty((128, B_COLS), dtype=fp8)
    for i, (k, j) in enumerate(B_BLOCKS):
        critB[:, i * 128 : (i + 1) * 128] = blk(k, j)
    critB = np.ascontiguousarray(critB)

    critC = np.empty((128, C_COLS), dtype=fp8)
    critC[:, :C_W2] = _chunk_pack(_q8(w1, S_W1), H)
    critC[:, C_W2:C_WP0] = _q8(w2, S_W2)
    critC[:, C_WP0 : C_WP0 + 128] = blk(0, 1)
    critC[:, C_WP0 + 128 :] = blk(1, 1)
    critC = np.ascontiguousarray(critC)

    f32_img = np.zeros((128, F32_COLS), dtype=np.float32)
    f32_img[:, :PK_B1] = _chunk_pack(fm_w * S_FMW, KFM)
    f32_img[:, PK_B1] = b1 * S_W1
    f32_img[:, PK_B2] = b2 * S_W1 * S_W2
    f32_img[:, PK_BO] = bo[0]
    bf_img = np.zeros((128, 2), dtype=bf)
    bf_img[:, 0] = (wo[NP:] * S_WOH).astype(bf)
    bf_img[:, 1] = bf(1.0)

    xT = np.ascontiguousarray(x.T)                       # [N, B] f32

    in_maps = []
    for c in range(N_CORES):
        critA = np.empty((128, A_COLS), dtype=fp8)
        critA[:, XT_OFF:F32_OFF] = _chunk_pack(
            _q8(xT[:, c * BC : (c + 1) * BC], 1.0), BC
        )
        critA[:, F32_OFF:BF_OFF] = f32_img.view(fp8)
        critA[:, BF_OFF:WPA_OFF] = bf_img.view(fp8)
        critA[:, WPA_OFF:] = blk(0, 0)
        in_maps.append(
            {
                "critA": np.ascontiguousarray(critA),
                "critB": critB,
                "critC": critC,
            }
        )
    return in_maps


def run(inputs, **spmd_kwargs):
    """Build, run on 8 cores, return (output [512,1] f32, BassKernelResults)."""
    nc = _build_program()
    in_maps = _prep_inputs(**inputs)
    res = run_bass_kernel_spmd(nc, in_maps, list(range(N_CORES)), **spmd_kwargs)
    out = np.concatenate(
        [res.results[c]["out"].reshape(BC) for c in range(N_CORES)]
    ).reshape(B, 1).astype(np.float32)
    return out, res


def kernel(**inputs) -> np.ndarray:
    out, _ = run(inputs)
    return out


# revision 14
# speedup vs baseline: 1.1741x; 1.0831x over previous
"""DeepFM forward on Trainium2, 8 NeuronCores, data-parallel over batch.

Reference computes (B=512, n=512, K=4, H=128, n_pairs=130816):
    S  = fm_w @ fm_w.T
    fm = x[:, i1] * x[:, i2] * S[i1, i2]        # [B, n_pairs]
    h2 = relu(relu(x@w1+b1)@w2+b2)
    out = sigmoid(concat([fm, h2]) @ wo + bo)

The fm @ wo[:n_pairs] contraction is the bilinear form
    t1[b] = x[b]^T Wp' x[b]  with  Wp'[i,j] = S[i,j] * Wp[i,j]
where Wp is wo[:n_pairs] scattered into the strictly-upper triangle of a
[n, n] matrix (host-side static relayout). S = fm_w fm_w^T has rank 4, so
    t1[b] = sum_t z_t[b]^T Wp z_t[b],  z_t = x * fm_w[:, t]
and only the 10 upper-triangular 128x128 blocks of Wp are shipped.

All matmul operands are fp8e4 (TRN e4m3, +-240) with power-of-2 scales
chosen so every tensor sits mid-range; the final sigmoid applies the
inverse scale. This halves HBM traffic vs bf16 (the DMA is the dominant
cost) with ~7e-4 relative error (threshold 2e-2).

Everything lives in the [batch, t] free layout so the final fold over t
is one Vector tensor_reduce. Per-core program (BC=64 batch cols):
    z_k[:, b, t] = xT_k * fmw16[k][:, t]        (DVE/Pool, fp8 out)
    VT_j = sum_{k<=j} Wp[k,j]^T @ z_k           (PE, fp8, psum f32)
    Q_j  = VT_j * z_j                           (DVE, bf16)
    t_ps[1, b, t] += ones^T @ Q_j               (PE)
    h1 = relu8(w1^T xT + 16 b1); h2 = relu16(w2^T h1 + 32 b2)  (PE+ACT)
    deep_ps = (2048 woh)^T @ h2                 (PE)
    t = reduce_t(t_ps); out = sigmoid((t + deep)/65536 + bo)   (DVE+ACT)

DMA plan (fixed ~700ns per dma_start on the issuing queue; transfers
share the SDMA rings): sync carries critA (x, fm_w, biases, wp00) then
critB (wp j2/j3 blocks); scalar carries critC (w1, w2, wp j1) in
parallel. PE is HAM-warmed with dummy fp8 matmuls during the DMA wait,
and a few post-output dummy ops keep the clock up into the teardown.
"""

import os
import sys

import numpy as np

for _p in ("/opt/trn_rl_repo", "/root/.axon_site/_ro/trn_rl_repo"):
    if os.path.isdir(_p) and _p not in sys.path:
        sys.path.insert(0, _p)

import ml_dtypes

import concourse.bass as bass
import concourse.tile as tile
from concourse import bacc, mybir
from concourse.bass_utils import run_bass_kernel_spmd

F32 = mybir.dt.float32
BF16 = mybir.dt.bfloat16
FP8 = mybir.dt.float8e4
AF = mybir.ActivationFunctionType
ALU = mybir.AluOpType

N = 512          # n_feat
KFM = 4          # fm embedding dim
H = 128          # mlp hidden
NP = N * (N - 1) // 2
B = 512
N_CORES = 8
BC = B // N_CORES  # 64 batch rows per core
NCH = N // 128     # 4 feature chunks

# fp8 scales (powers of two)
S_FMW = 16.0
S_WP = 256.0
S_W1 = 16.0
S_W2 = 2.0
S_T = S_FMW * S_FMW * S_WP          # 65536 on both t contributions
S_WOH = S_T / (S_W1 * S_W2)         # 2048

N_WARM = int(os.environ.get("DFM_N_WARM", "8"))
WARM_COLS = int(os.environ.get("DFM_WARM_COLS", "512"))
RELU_ENG = os.environ.get("DFM_RELU_ENG", "scalar")
Z3_ENG = os.environ.get("DFM_Z3_ENG", "gpsimd")
HAM_TAIL = int(os.environ.get("DFM_HAM_TAIL", "2"))
B_ENG = os.environ.get("DFM_B_ENG", "sync")      # sync | scalar | gpsimd
DEEP_SLOT = os.environ.get("DFM_DEEP_SLOT", "1") == "1"

# critA image (fp8 bytes): [ xT (4*64) | f32 pack (19*4) | woh bf16 | ones bf16 | wp(0,0) ]
XT_OFF = 0
F32_OFF = NCH * BC               # 256
F32_COLS = NCH * KFM + 3         # fmw16 | b1*16 | b2*32 | bo  -> 19
BF_OFF = F32_OFF + F32_COLS * 4  # 332
WPA_OFF = BF_OFF + 4             # 336
A_COLS = WPA_OFF + 128           # 464
PK_B1 = NCH * KFM                # 16
PK_B2 = PK_B1 + 1
PK_BO = PK_B2 + 1

# critC: [ w1*16 (4*128) | w2*2 (128) | wp(0,1) | wp(1,1) ]
C_W2 = NCH * H                   # 512
C_WP0 = C_W2 + H                 # 640
C_COLS = C_WP0 + 2 * 128         # 896

# critB: wp blocks (0,2),(1,2),(2,2),(0,3),(1,3),(2,3),(3,3)
B_BLOCKS = [(0, 2), (1, 2), (2, 2), (0, 3), (1, 3), (2, 3), (3, 3)]
B_COLS = len(B_BLOCKS) * 128     # 896

_IU1, _IU2 = np.triu_indices(N, k=1)

_program_cache = None


def _build_program():
    global _program_cache
    if _program_cache is not None:
        return _program_cache

    nc = bacc.Bacc(
        "TRN2", target_bir_lowering=False, debug=False, num_devices=N_CORES
    )
    critA_d = nc.declare_dram_parameter("critA", [128, A_COLS], FP8, isOutput=False)
    critB_d = nc.declare_dram_parameter("critB", [128, B_COLS], FP8, isOutput=False)
    critC_d = nc.declare_dram_parameter("critC", [128, C_COLS], FP8, isOutput=False)
    out_d = nc.declare_dram_parameter("out", [1, BC], F32, isOutput=True)

    relu_eng_is_scalar = RELU_ENG == "scalar"

    with tile.TileContext(nc) as tc:
        with (
            tc.tile_pool(name="const", bufs=1) as cpool,
            tc.tile_pool(name="work", bufs=1) as wpool,
            tc.tile_pool(name="ps_v", bufs=1, space=bass.MemorySpace.PSUM) as vpool,
            tc.tile_pool(name="ps_h", bufs=1, space=bass.MemorySpace.PSUM) as hpool,
            tc.tile_pool(name="ps_t", bufs=1, space=bass.MemorySpace.PSUM) as tpool,
        ):
            # ---- input DMAs: sync gets A then B, scalar gets C ----
            critA_sb = cpool.tile([128, A_COLS], FP8)
            nc.sync.dma_start(critA_sb[:], critA_d[:, :])
            critC_sb = cpool.tile([128, C_COLS], FP8)
            nc.scalar.dma_start(critC_sb[:], critC_d[:, :])
            critB_sb = cpool.tile([128, B_COLS], FP8)
            b_eng = {"sync": nc.sync, "scalar": nc.scalar, "gpsimd": nc.gpsimd}[B_ENG]
            b_eng.dma_start(critB_sb[:], critB_d[:, :])

            # ---- views into critA ----
            f32v = critA_sb[:, F32_OFF:BF_OFF].bitcast(F32)       # [128, 19]
            bf16v = critA_sb[:, BF_OFF:WPA_OFF].bitcast(BF16)     # [128, 2]
            woh_ap = bf16v[:, 0:1]
            ones_ap = bf16v[:, 1:2]
            b1_ap = f32v[:, PK_B1 : PK_B1 + 1]
            b2_ap = f32v[:, PK_B2 : PK_B2 + 1]
            bo_ap = f32v[0:1, PK_BO : PK_BO + 1]

            def xt(k):
                return critA_sb[:, XT_OFF + k * BC : XT_OFF + (k + 1) * BC]

            def fmw(k):
                return f32v[:, k * KFM : (k + 1) * KFM]

            wpA = critA_sb[:, WPA_OFF : WPA_OFF + 128]

            def w1c(k):
                return critC_sb[:, k * H : (k + 1) * H]

            w2_ap = critC_sb[:, C_W2 : C_W2 + H]
            wp_blk = {(0, 0): wpA}
            wp_blk[(0, 1)] = critC_sb[:, C_WP0 : C_WP0 + 128]
            wp_blk[(1, 1)] = critC_sb[:, C_WP0 + 128 : C_WP0 + 256]
            for i, kj in enumerate(B_BLOCKS):
                wp_blk[kj] = critB_sb[:, i * 128 : (i + 1) * 128]

            # ---- warm tiles (GpSimd memsets; Vector stays free) ----
            dum_lhs = cpool.tile([128, 128], FP8)
            nc.gpsimd.memset(dum_lhs[:], 0.0)
            dum_rhs = cpool.tile([128, WARM_COLS], FP8)
            nc.gpsimd.memset(dum_rhs[:], 0.0)
            warm_in = cpool.tile([1, 1], F32)
            nc.gpsimd.memset(warm_in[:], 0.0)

            warm_cols = WARM_COLS if DEEP_SLOT else BC
            warm_ps = hpool.tile(
                [128, warm_cols], F32, tag="warm" if DEEP_SLOT else "h1"
            )
            for d in range(N_WARM):
                nc.tensor.matmul(
                    warm_ps[:], dum_lhs[:], dum_rhs[:, :warm_cols],
                    start=True, stop=True,
                )
            warm_out = cpool.tile([1, 1], F32)
            nc.scalar.activation(warm_out[:], warm_in[:], AF.Sigmoid, bias=0.0)

            # ---- z_k [128, BC, KFM] fp8: x column-scaled by fm_w (rank-4) ----
            z_all = wpool.tile([128, NCH, BC, KFM], FP8, name="z_all", tag="z")

            def z(k):
                return z_all[:, k]

            for k in range(NCH):
                eng = nc.vector if (k < 3 or Z3_ENG == "vector") else nc.gpsimd
                eng.tensor_mul(
                    z(k),
                    xt(k)[:, :, None].broadcast_to([128, BC, KFM]),
                    fmw(k)[:, None, :].broadcast_to([128, BC, KFM]),
                )

            # ---- psum tiles ----
            vt = [
                vpool.tile([128, BC, KFM], F32, name=f"vt{j}", tag=f"v{j}")
                for j in range(NCH)
            ]
            h1_ps = hpool.tile([H, BC], F32, tag="h1")
            h2_ps = hpool.tile([H, BC], F32, tag="h2")
            t_ps = tpool.tile([1, BC, KFM], F32, tag="t")
            deep_out = (
                t_ps[:, :, 0:1] if DEEP_SLOT else tpool.tile([1, BC], F32, tag="deep")[:]
            )

            q_all = wpool.tile([128, NCH, BC, KFM], BF16, name="q_all", tag="q")

            def q(j):
                return q_all[:, j]

            h1_sb = wpool.tile([H, BC], FP8, name="h1_sb")
            h2_sb = wpool.tile([H, BC], BF16, name="h2_sb")

            def relu(dst, src, bias_ap):
                if relu_eng_is_scalar:
                    nc.scalar.activation(dst, src, AF.Relu, bias=bias_ap)
                else:
                    nc.vector.tensor_scalar(
                        dst, src, bias_ap, 0.0, op0=ALU.add, op1=ALU.max
                    )

            # ---- PE stream interleaved with DVE/ACT consumers ----
            # VT j0 (needs critA only)
            nc.tensor.matmul(vt[0][:], wpA, z(0), start=True, stop=True)
            # h1 (needs critC)
            for k in range(NCH):
                nc.tensor.matmul(
                    h1_ps[:], w1c(k), xt(k), start=(k == 0), stop=(k == NCH - 1)
                )
            relu(h1_sb[:], h1_ps[:], b1_ap)
            # VT j1
            nc.tensor.matmul(vt[1][:], wp_blk[(0, 1)], z(0), start=True, stop=False)
            nc.tensor.matmul(vt[1][:], wp_blk[(1, 1)], z(1), start=False, stop=True)
            # Q0
            nc.vector.tensor_mul(q(0), vt[0][:], z(0))
            # h2
            nc.tensor.matmul(h2_ps[:], w2_ap, h1_sb[:], start=True, stop=True)
            relu(h2_sb[:], h2_ps[:], b2_ap)
            # VT j2
            for i, k in enumerate(range(3)):
                nc.tensor.matmul(
                    vt[2][:], wp_blk[(k, 2)], z(k),
                    start=(i == 0), stop=(i == 2), skip_group_check=True,
                )
            # Q1
            nc.vector.tensor_mul(q(1), vt[1][:], z(1))
            # t accumulation group start + deep
            nc.tensor.matmul(
                t_ps[:], ones_ap, q(0), start=True, stop=False,
                skip_group_check=True,
            )
            nc.tensor.matmul(
                deep_out, woh_ap, h2_sb[:],
                start=not DEEP_SLOT, stop=not DEEP_SLOT,
                skip_group_check=True,
            )
            # VT j3
            for i, k in enumerate(range(4)):
                nc.tensor.matmul(
                    vt[3][:], wp_blk[(k, 3)], z(k),
                    start=(i == 0), stop=(i == 3), skip_group_check=True,
                )
            # Q2, t1, Q3, t2, t3
            nc.vector.tensor_mul(q(2), vt[2][:], z(2))
            nc.tensor.matmul(
                t_ps[:], ones_ap, q(1), start=False, stop=False,
                skip_group_check=True,
            )
            nc.vector.tensor_mul(q(3), vt[3][:], z(3))
            nc.tensor.matmul(
                t_ps[:], ones_ap, q(2), start=False, stop=False,
                skip_group_check=True,
            )
            nc.tensor.matmul(
                t_ps[:], ones_ap, q(3), start=False, stop=True,
                skip_group_check=True,
            )

            # ---- fold over t (deep already in slot 0), sigmoid, store ----
            tlog_sb = wpool.tile([1, BC], F32, name="tlog")
            nc.vector.tensor_reduce(
                tlog_sb[:], t_ps[:], axis=mybir.AxisListType.X, op=ALU.add
            )
            if not DEEP_SLOT:
                tlog2 = wpool.tile([1, BC], F32, name="tlog2")
                nc.vector.tensor_add(tlog2[:], tlog_sb[:], deep_out)
                tlog_sb = tlog2
            out_sb = wpool.tile([1, BC], F32, name="out_sb")
            nc.scalar.activation(
                out_sb[:], tlog_sb[:], AF.Sigmoid, bias=bo_ap, scale=1.0 / S_T
            )
            nc.sync.dma_start(out_d[:, :], out_sb[:])

            # ---- keep HAM clock high into the teardown sweep ----
            for _ in range(HAM_TAIL):
                nc.tensor.matmul(
                    warm_ps[:], dum_lhs[:], dum_rhs[:, :warm_cols],
                    start=True, stop=True,
                )

    nc.compile()
    _program_cache = nc
    return nc


def _q8(a, scale):
    return np.clip(
        np.asarray(a, np.float32) * scale, -240.0, 240.0
    ).astype(ml_dtypes.float8_e4m3fn)


def _chunk_pack(a, cols):
    """[512, cols] row-major -> [128, 4, cols] -> [128, 4*cols]."""
    return np.ascontiguousarray(
        a.reshape(NCH, 128, cols).transpose(1, 0, 2).reshape(128, NCH * cols)
    )


def _prep_inputs(x, fm_w, w1, b1, w2, b2, wo, bo):
    x = np.asarray(x, dtype=np.float32)
    fm_w = np.asarray(fm_w, dtype=np.float32)
    w1 = np.asarray(w1, dtype=np.float32)
    w2 = np.asarray(w2, dtype=np.float32)
    wo = np.asarray(wo, dtype=np.float32).reshape(NP + H)
    b1 = np.asarray(b1, dtype=np.float32).reshape(H)
    b2 = np.asarray(b2, dtype=np.float32).reshape(H)
    bo = np.asarray(bo, dtype=np.float32).reshape(1)

    fp8 = ml_dtypes.float8_e4m3fn
    bf = ml_dtypes.bfloat16

    # Pair weights scattered into the strictly-upper triangle, fp8-scaled.
    wp = np.zeros((N, N), dtype=np.float32)
    wp[_IU1, _IU2] = wo[:NP]
    wp_q = _q8(wp, S_WP)

    def blk(k, j):
        return wp_q[128 * k : 128 * (k + 1), 128 * j : 128 * (j + 1)]

    critB = np.empty((128, B_COLS), dtype=fp8)
    for i, (k, j) in enumerate(B_BLOCKS):
        critB[:, i * 128 : (i + 1) * 128] = blk(k, j)
    critB = np.ascontiguousarray(critB)

    critC = np.empty((128, C_COLS), dtype=fp8)
    critC[:, :C_W2] = _chunk_pack(_q8(w1, S_W1), H)
    critC[:, C_W2:C_WP0] = _q8(w2, S_W2)
    critC[:, C_WP0 : C_WP0 + 128] = blk(0, 1)
    critC[:, C_WP0 + 128 :] = blk(1, 1)
    critC = np.ascontiguousarray(critC)

    f32_img = np.zeros((128, F32_COLS), dtype=np.float32)
    f32_img[:, :PK_B1] = _chunk_pack(fm_w * S_FMW, KFM)
    f32_img[:, PK_B1] = b1 * S_W1
    f32_img[:, PK_B2] = b2 * S_W1 * S_W2
    f32_img[:, PK_BO] = bo[0]
    bf_img = np.zeros((128, 2), dtype=bf)
    bf_img[:, 0] = (wo[NP:] * S_WOH).astype(bf)
    bf_img[:, 1] = bf(1.0)

    xT = np.ascontiguousarray(x.T)                       # [N, B] f32

    in_maps = []
    for c in range(N_CORES):
        critA = np.empty((128, A_COLS), dtype=fp8)
        critA[:, XT_OFF:F32_OFF] = _chunk_pack(
            _q8(xT[:, c * BC : (c + 1) * BC], 1.0), BC
        )
        critA[:, F32_OFF:BF_OFF] = f32_img.view(fp8)
        critA[:, BF_OFF:WPA_OFF] = bf_img.view(fp8)
        critA[:, WPA_OFF:] = blk(0, 0)
        in_maps.append(
            {
                "critA": np.ascontiguousarray(critA),
                "critB": critB,
                "critC": critC,
            }
        )
    return in_maps


def run(inputs, **spmd_kwargs):
    """Build, run on 8 cores, return (output [512,1] f32, BassKernelResults)."""
    nc = _build_program()
    in_maps = _prep_inputs(**inputs)
    res = run_bass_kernel_spmd(nc, in_maps, list(range(N_CORES)), **spmd_kwargs)
    out = np.concatenate(
        [res.results[c]["out"].reshape(BC) for c in range(N_CORES)]
    ).reshape(B, 1).astype(np.float32)
    return out, res


def kernel(**inputs) -> np.ndarray:
    out, _ = run(inputs)
    return out
